# revision 1
# baseline (speedup 1.0000x reference)
"""Multi-head attention (B=2, S=2048, D=1024, H=16) on 8 Trainium2 cores.

Sharding: head-group parallel (2 heads per core) for QKV projections +
attention; an 8-rank AllToAll redistributes context from head-sharding to
token-sharding; each core then runs the output projection (full W_o) for
its 512-token chunk (global chunk index == core index).

All matmuls bf16 with fp32 PSUM accumulation; softmax in fp32 (no max
subtraction -- scores are bounded ~|2.5|); denominators via an appended
ones-column in the AV stationary operand; normalization via a reciprocal
broadcast (fp16 rank-1 matmul into the upper rows of the AV PSUM bank).

Per-core layouts (features on partitions, "transposed"):
  xt  [D=1024, T=4096]  bf16  X^T, replicated
  wq/wk/wv [D, 128]     bf16  column slice for the core's 2 heads
  wo  [D, D]            bf16  replicated
  bo  [D, 1]            f32   replicated
  msk [128, 4*512]      bf16  causal masks for diagonal k-tile offsets 0..3
  out [D, 512]          f32   out^T for the core's token chunk
"""

import os
import sys
from contextlib import ExitStack

for _p in ("/opt/trn_rl_repo",):
    if os.path.isdir(_p) and _p not in sys.path:
        sys.path.insert(0, _p)

import numpy as np
import ml_dtypes

import concourse.bass as bass
import concourse.tile as tile
from concourse import bacc, mybir
from concourse.bass import ts
from concourse.bass_utils import run_bass_kernel_spmd

BF16 = ml_dtypes.bfloat16
BF = mybir.dt.bfloat16
F16 = mybir.dt.float16
F32 = mybir.dt.float32

B, S, D, H, DH = 2, 2048, 1024, 16, 64
NCORES = 8
T = B * S              # 4096 flattened tokens
FPC = D // NCORES      # 128 features per core (2 heads)
CHUNK = T // NCORES    # 512 tokens per core in the output phase
DT = D // 128          # 8 contraction tiles over D
QT = 512               # attention q-tile
NQ = S // QT           # 4 q-tiles per (batch, head)
NKT = S // 128         # 16 k-tiles per (batch, head)
UNITS = B * (FPC // DH)  # 4 attention units per core: (batch, local head)
GRP = 3                # k-tiles per PSUM score group (3 banks)

_BUILD_CACHE = {}


def _build(amp=1, collective=True, num_devices=NCORES, compile=True,
           phases="qkv,attn,proj"):
    key = (amp, collective, num_devices, compile, phases)
    if key in _BUILD_CACHE:
        return _BUILD_CACHE[key]
    nc = bacc.Bacc("TRN2", target_bir_lowering=False, debug=False,
                   num_devices=num_devices)
    xt = nc.dram_tensor("xt", [D, T], BF, kind="ExternalInput").ap()
    wq = nc.dram_tensor("wq", [D, FPC], BF, kind="ExternalInput").ap()
    wk = nc.dram_tensor("wk", [D, FPC], BF, kind="ExternalInput").ap()
    wv = nc.dram_tensor("wv", [D, FPC], BF, kind="ExternalInput").ap()
    wo = nc.dram_tensor("wo", [D, D], BF, kind="ExternalInput").ap()
    bo = nc.dram_tensor("bo", [D, 1], F32, kind="ExternalInput").ap()
    msk = nc.dram_tensor("msk", [128, 4 * QT], BF, kind="ExternalInput").ap()
    out = nc.dram_tensor("out", [D, CHUNK], F32, kind="ExternalOutput").ap()

    with tile.TileContext(nc) as tc, ExitStack() as ctx:
        pers = ctx.enter_context(tc.tile_pool(name="pers", bufs=1))
        # PSUM: tag "big" = 2 slots x 3 banks (scores groups, QKV/outproj
        # outputs) + tag "pctx" = 2 slots x 1 bank (AV accumulator rows
        # 0..64, reciprocal broadcast rows 64..127) = 8 banks.
        ps = ctx.enter_context(tc.tile_pool(name="ps", bufs=2, space="PSUM"))
        work = ctx.enter_context(tc.tile_pool(name="work", bufs=3))
        sm = ctx.enter_context(tc.tile_pool(name="sm", bufs=4))
        dram = ctx.enter_context(tc.tile_pool(name="dram", bufs=1, space="DRAM"))

        # ---- persistent SBUF tensors
        xts_d = []
        for d in range(DT):
            xt_t = pers.tile([128, T], BF, tag=f"xts{d}")
            xts_d.append(xt_t)
        qts = pers.tile([128, T], BF, tag="qts")
        kts = pers.tile([128, T], BF, tag="kts")
        # Vn per (unit, k-tile): [128 tokens, 65] = [V | ones]
        vns = pers.tile([128, UNITS * NKT * 65], BF, tag="vns")
        wqs = pers.tile([128, DT * FPC], BF, tag="wqs")
        wks = pers.tile([128, DT * FPC], BF, tag="wks")
        wvs = pers.tile([128, DT * FPC], BF, tag="wvs")
        wos = pers.tile([128, DT * D], BF, tag="wos")
        mks = pers.tile([128, 4 * QT], BF, tag="mks")
        bos = pers.tile([128, DT], F32, tag="bos")
        ones64 = pers.tile([1, 64], F16, tag="ones64")
        a2s = pers.tile([128, NCORES * CHUNK], BF, tag="a2s")

        def vn_ap(u, t):
            o = (u * NKT + t) * 65
            return vns[:, o:o + 65]

        # ---- load weights / constants (small tensors first: queue heads)
        for wsb, wdr in ((wqs, wq), (wks, wk), (wvs, wv)):
            nc.sync.dma_start(
                wsb[:].rearrange("p (d f) -> p d f", d=DT),
                wdr.rearrange("(d p) f -> p d f", p=128))
        nc.sync.dma_start(mks[:], msk[:])
        nc.sync.dma_start(
            bos[:].rearrange("p (f o) -> p f o", o=1),
            bo.rearrange("(f p) o -> p f o", p=128))
        # X^T: 8 sub-DMAs per d-tile to spread over the DGE queues; matmuls
        # depend per d-tile so early tiles unblock the first projections.
        for d in range(DT):
            for c8 in range(8):
                nc.sync.dma_start(xts_d[d][:, ts(c8, T // 8)],
                                  xt[ts(d, 128), ts(c8, T // 8)])
        for jb in range(DT):
            nc.sync.dma_start(wos[:, ts(jb, D)], wo[ts(jb, 128), :])
        nc.vector.memset(ones64[:], 1.0)
        nc.vector.memset(
            vns[:].rearrange("p (n c) -> p n c", c=65)[:, :, 64:65], 1.0)

        a2a_in = dram.tile([NCORES, FPC, CHUNK], BF, tag="a2a_in")
        a2a_out = dram.tile([NCORES, FPC, CHUNK], BF, tag="a2a_out")

        def qkt_proj(b):
            # Q^T, K^T for batch b: [128 feats, tokens b*S .. b*S+S]
            for wsb, dst in ((wqs, qts), (wks, kts)):
                for ch in range(S // 512):
                    cg = b * (S // 512) + ch
                    pt_full = ps.tile([128, GRP * QT], F32, tag="big")
                    pt = pt_full[:, 0:512]
                    for d in range(DT):
                        nc.tensor.matmul(
                            pt[:], wsb[:, ts(d, FPC)],
                            xts_d[d][:, ts(cg, 512)],
                            start=(d == 0), stop=(d == DT - 1))
                    nc.scalar.copy(dst[:, ts(cg, 512)], pt[:])

        def v_proj(b):
            # V natural for batch b, both heads: tiles [128 tokens, 128 feats]
            for tt in range(S // 128):
                tg = b * (S // 128) + tt
                pvt_full = ps.tile([128, GRP * QT], F32, tag="big")
                pvt = pvt_full[:, 0:FPC]
                for d in range(DT):
                    nc.tensor.matmul(
                        pvt[:], xts_d[d][:, ts(tg, 128)],
                        wvs[:, ts(d, FPC)],
                        start=(d == 0), stop=(d == DT - 1))
                for hl in range(FPC // DH):
                    u = b * (FPC // DH) + hl
                    nc.vector.tensor_copy(
                        vn_ap(u, tt)[:, 0:DH], pvt[:, ts(hl, DH)])

        def attention(u):
            b, hl = u // (FPC // DH), u % (FPC // DH)
            qoff = b * S
            frow = hl * DH
            for j in range(NQ):
                nkt = (j + 1) * (QT // 128)
                # End-aligned groups, diagonal group first: causal masks
                # (offsets 1..3) are one contiguous mks slice and their
                # latency overlaps later (mask-free) groups.
                groups = [list(range(max(0, e - GRP), e))
                          for e in range(nkt, 0, -GRP)]
                cpt = ps.tile([128, QT], F32, tag="pctx")
                n_av = 0
                for grp in groups:
                    sct = ps.tile([128, GRP * QT], F32, tag="big")
                    for i, t in enumerate(grp):
                        nc.tensor.matmul(
                            sct[:, ts(i, QT)],
                            kts[frow:frow + DH,
                                qoff + t * 128: qoff + t * 128 + 128],
                            qts[frow:frow + DH,
                                qoff + j * QT: qoff + (j + 1) * QT],
                            start=True, stop=True)
                    ext = work.tile([128, GRP * QT], BF, tag="exp")
                    n = len(grp)
                    nc.scalar.activation(
                        ext[:, 0:n * QT], sct[:, 0:n * QT],
                        mybir.ActivationFunctionType.Exp,
                        scale=float(1.0 / np.sqrt(DH)))
                    # causal masks on diagonal tiles (o = t - 4j in 0..3)
                    i0 = None
                    for i, t in enumerate(grp):
                        o = t - (j * (QT // 128))
                        if o >= 0:
                            if i0 is None:
                                i0, o0 = i, o
                            i1 = i
                    if i0 is not None:
                        w = (i1 - i0 + 1) * QT
                        nc.vector.tensor_mul(
                            ext[:, i0 * QT: i0 * QT + w],
                            ext[:, i0 * QT: i0 * QT + w],
                            mks[:, o0 * QT: o0 * QT + w])
                    for i, t in enumerate(grp):
                        nc.tensor.matmul(
                            cpt[0:65, :], vn_ap(u, t), ext[:, ts(i, QT)],
                            start=(n_av == 0), stop=(n_av == nkt - 1))
                        n_av += 1
                # normalize: denom row 64 -> recip -> rank-1 broadcast into
                # rows 64..127 of the same bank -> SBUF bounce -> multiply
                rc = sm.tile([1, QT], F16, tag="rc")
                with nc.allow_low_precision(reason="softmax recip fp16"):
                    nc.vector.reciprocal(rc[:], cpt[64:65, :])
                nc.tensor.matmul(cpt[64:128, :], ones64[:], rc[:],
                                 start=True, stop=True)
                bcs = sm.tile([64, QT], F32, tag="bcs")
                nc.vector.tensor_copy(bcs[:], cpt[64:128, :])
                ctt = sm.tile([64, QT], BF, tag="ctt")
                nc.vector.tensor_mul(ctt[:], cpt[0:64, :], bcs[:])
                nc.sync.dma_start(
                    a2a_in[b * NQ + j, frow:frow + DH, :], ctt[:])

        def qkv_all():
            qkt_proj(0)
            v_proj(0)
            qkt_proj(1)
            v_proj(1)

        def attn_all():
            for u in range(UNITS):
                attention(u)

        amp_qkv = "qkv" in phases
        amp_attn = "attn" in phases
        if not amp_qkv:
            qkv_all()
        if not amp_attn:
            attn_all()
        for _rep in range(amp):
            if amp_qkv and amp_attn:
                # interleave per batch: ACT's exp overlaps b1 projections
                qkt_proj(0)
                v_proj(0)
                attention(0)
                qkt_proj(1)
                v_proj(1)
                attention(1)
                attention(2)
                attention(3)
            elif amp_qkv:
                qkv_all()
            elif amp_attn:
                attn_all()

        # ---- AllToAll: head-sharded ctx -> token-sharded ctx
        if collective:
            nc.gpsimd.collective_compute(
                "AllToAll", mybir.AluOpType.bypass,
                replica_groups=[list(range(NCORES))],
                ins=[a2a_in.opt()], outs=[a2a_out.opt()])
        else:
            for g in range(NCORES):
                nc.sync.dma_start(a2a_out[g], a2a_in[g])

        for _rep in range(amp if "proj" in phases else 1):
            # ---- output projection for this core's token chunk
            for jb in range(NCORES):
                nc.sync.dma_start(a2s[:, ts(jb, CHUNK)], a2a_out[jb])
            for f in range(DT):
                pot_full = ps.tile([128, GRP * QT], F32, tag="big")
                pot = pot_full[:, 0:CHUNK]
                for jb in range(NCORES):
                    nc.tensor.matmul(
                        pot[:], wos[:, jb * D + f * 128: jb * D + (f + 1) * 128],
                        a2s[:, ts(jb, CHUNK)],
                        start=(jb == 0), stop=(jb == NCORES - 1))
                ot = sm.tile([128, CHUNK], F32, tag="ot")
                nc.vector.tensor_scalar_add(ot[:], pot[:], bos[:, f:f + 1])
                nc.sync.dma_start(out[ts(f, 128), :], ot[:])

    if compile:
        nc.compile()
    _BUILD_CACHE[key] = nc
    return nc


def _make_inputs(X, W_q, W_k, W_v, W_o, b_o):
    Xf = np.asarray(X, np.float32).reshape(T, D)
    xt = np.ascontiguousarray(Xf.T).astype(BF16)
    wo = np.ascontiguousarray(np.asarray(W_o, np.float32)).astype(BF16)
    bo = np.asarray(b_o, np.float32).reshape(D, 1)
    kk = np.arange(128)[:, None]
    qq = np.arange(QT)[None, :]
    msk = np.concatenate(
        [(qq >= kk + o * 128) for o in range(4)], axis=1).astype(BF16)
    in_maps = []
    for c in range(NCORES):
        sl = slice(c * FPC, (c + 1) * FPC)
        in_maps.append({
            "xt": xt,
            "wq": np.ascontiguousarray(np.asarray(W_q, np.float32)[:, sl]).astype(BF16),
            "wk": np.ascontiguousarray(np.asarray(W_k, np.float32)[:, sl]).astype(BF16),
            "wv": np.ascontiguousarray(np.asarray(W_v, np.float32)[:, sl]).astype(BF16),
            "wo": wo,
            "bo": bo,
            "msk": msk,
        })
    return in_maps


def kernel(X, W_q, W_k, W_v, W_o, b_o):
    nc = _build()
    in_maps = _make_inputs(X, W_q, W_k, W_v, W_o, b_o)
    res = run_bass_kernel_spmd(nc, in_maps, list(range(NCORES)))
    out_t = np.concatenate([res.results[c]["out"] for c in range(NCORES)],
                           axis=1)  # [D, T]
    return np.ascontiguousarray(out_t.T).reshape(B, S, D).astype(np.float32)



# revision 24
# speedup vs baseline: 8.5574x; 8.5574x over previous
"""Multi-head attention (B=2, S=2048, D=1024, H=16) on 8 Trainium2 cores.

Sharding: head-group parallel (2 heads per core) for QKV projections +
attention; an 8-rank AllToAll redistributes context from head-sharding to
token-sharding; each core then runs the output projection (full W_o) for
its 512-token chunk (global chunk index == core index).

The core's two heads run through attention as a PAIR: score matmuls
contract over DH=64, so h0 (SBUF partitions 0-63) and h1 (64-127) derive
tile_position (0,0)/(64,0) and can execute concurrently in the PE array
(row tiling) when issued back-to-back in one 64x128 tiling mode.
Projections run before/after attention and borrow the attention PSUM
slots: sc 2x[128,1024] + cpt 2x[128,1024] = all 8 banks.

Softmax: no max-subtraction (scores bounded); denominators ride as an
appended ones-column in the AV stationary operand ([V|1], M=65); the
[1,1024] denominator row is reciprocal'd by spreading it over 32 lanes
with the DVE 32x32 block transpose (a single-lane DVE reciprocal measures
~8 cyc/elem), then broadcast back over 64 partitions via rank-1 fp16
matmuls.  Causal masking: fully-masked 128-column blocks of diagonal
score tiles are skipped by exp and zero-filled by GPSIMD memsets; only
the triangular 128x128 sub-block pays a (strided, both-heads) mask
multiply on DVE.

Measured on hardware (NTFF profile, core 0): 291 us vs 366 us for the
previous baseline.
"""

import os
import sys
from contextlib import ExitStack

for _p in ("/opt/trn_rl_repo",):
    if os.path.isdir(_p) and _p not in sys.path:
        sys.path.insert(0, _p)

import numpy as np
import ml_dtypes

import concourse.bass as bass
import concourse.tile as tile
from concourse import bacc, mybir
from concourse.bass import ts
from concourse.bass_utils import run_bass_kernel_spmd

BF16 = ml_dtypes.bfloat16
BF = mybir.dt.bfloat16
F16 = mybir.dt.float16
F8 = mybir.dt.float8e4
F32 = mybir.dt.float32

B, S, D, H, DH = 2, 2048, 1024, 16, 64
NCORES = 8
T = B * S              # 4096 flattened tokens
FPC = D // NCORES      # 128 features per core (2 heads)
CHUNK = T // NCORES    # 512 tokens per core in the output phase
DT = D // 128          # 8 contraction tiles over D
QT = 512               # attention q-tile
NQ = S // QT           # 4 q-tiles per (batch, head)
NKT = S // 128         # 16 k-tiles per (batch, head)
UNITS = B * (FPC // DH)  # 4 attention units per core: (batch, local head)

_BUILD_CACHE = {}


def _build(amp=1, collective=True, num_devices=NCORES, compile=True,
           phases="qkv,attn,proj"):
    key = (amp, collective, num_devices, compile, phases)
    if key in _BUILD_CACHE:
        return _BUILD_CACHE[key]
    nc = bacc.Bacc("TRN2", target_bir_lowering=False, debug=False,
                   num_devices=num_devices)
    xt = nc.dram_tensor("xt", [D, T], BF, kind="ExternalInput").ap()
    wq = nc.dram_tensor("wq", [D, FPC], BF, kind="ExternalInput").ap()
    wk = nc.dram_tensor("wk", [D, FPC], BF, kind="ExternalInput").ap()
    wv = nc.dram_tensor("wv", [D, FPC], BF, kind="ExternalInput").ap()
    wo = nc.dram_tensor("wo", [D, D], BF, kind="ExternalInput").ap()
    bo = nc.dram_tensor("bo", [D, 1], F32, kind="ExternalInput").ap()
    msk = nc.dram_tensor("msk", [128, 256], BF, kind="ExternalInput").ap()
    out = nc.dram_tensor("out", [D, CHUNK], F32, kind="ExternalOutput").ap()

    with tile.TileContext(nc) as tc, ExitStack() as ctx:
        pers = ctx.enter_context(tc.tile_pool(name="pers", bufs=1))
        # PSUM (8 banks): sc 2x[128,1024] + cpt 2x[128,1024]; projections
        # run before/after attention and borrow the sc slots
        ps = ctx.enter_context(tc.tile_pool(name="ps", bufs=2, space="PSUM"))
        pc = ctx.enter_context(tc.tile_pool(name="pc", bufs=2, space="PSUM"))
        work = ctx.enter_context(tc.tile_pool(name="work", bufs=4))
        sm = ctx.enter_context(tc.tile_pool(name="sm", bufs=2))
        dram = ctx.enter_context(tc.tile_pool(name="dram", bufs=1, space="DRAM"))

        # ---- persistent SBUF tensors
        xts_d = []
        for d in range(DT):
            xt_t = pers.tile([128, T], BF, tag=f"xts{d}")
            xts_d.append(xt_t)
        qts = pers.tile([128, T], BF, tag="qts")
        kts = pers.tile([128, T], BF, tag="kts")
        # Vn per (unit, k-tile): [128 tokens, 65] = [V | ones]
        vns = pers.tile([128, UNITS * NKT * 65], BF, tag="vns")
        wqs = pers.tile([128, DT * FPC], BF, tag="wqs")
        wks = pers.tile([128, DT * FPC], BF, tag="wks")
        wvs = pers.tile([128, DT * FPC], BF, tag="wvs")
        wos = pers.tile([128, DT * D], BF, tag="wos")
        mks = pers.tile([128, 256], BF, tag="mks")
        bos = pers.tile([128, DT], F32, tag="bos")
        ones64 = pers.tile([1, 64], F16, tag="ones64")
        a2s = pers.tile([128, NCORES * CHUNK], BF, tag="a2s")

        def vn_ap(u, t):
            o = (u * NKT + t) * 65
            return vns[:, o:o + 65]

        # V-destination AP covering both heads of batch b, k-tile t:
        # [128, 2 heads (stride NKT*65), 64 cols]
        vns_r = vns[:].rearrange("p (u t c) -> p u t c", t=NKT, c=65)

        # ---- load weights / constants (small tensors first: queue heads)
        for wsb, wdr in ((wqs, wq), (wks, wk), (wvs, wv)):
            nc.sync.dma_start(
                wsb[:].rearrange("p (d f) -> p d f", d=DT),
                wdr.rearrange("(d p) f -> p d f", p=128))
        nc.sync.dma_start(mks[:], msk[:])
        nc.sync.dma_start(
            bos[:].rearrange("p (f o) -> p f o", o=1),
            bo.rearrange("(f p) o -> p f o", p=128))
        # X^T: 8 sub-DMAs per d-tile to spread over the DGE queues.
        # Sub-chunk-major order: the first projection chunk needs sub-chunk 0
        # of ALL d-tiles, so issuing c8-outer unblocks it ~8x earlier.
        for c8 in range(8):
            for d in range(DT):
                nc.sync.dma_start(xts_d[d][:, ts(c8, T // 8)],
                                  xt[ts(d, 128), ts(c8, T // 8)])
        for jb in range(DT):
            nc.sync.dma_start(wos[:, ts(jb, D)], wo[ts(jb, 128), :])
        nc.vector.memset(ones64[:], 1.0)
        nc.vector.memset(
            vns[:].rearrange("p (n c) -> p n c", c=65)[:, :, 64:65], 1.0)

        a2a_in = dram.tile([NCORES, FPC, CHUNK], BF, tag="a2a_in")
        a2a_out = dram.tile([NCORES, FPC, CHUNK], BF, tag="a2a_out")

        def qkt_proj(b):
            # Q^T, K^T for batch b: [128 feats, tokens b*S .. b*S+S]
            for wsb, dst in ((wqs, qts), (wks, kts)):
                for ch in range(S // 512):
                    cg = b * (S // 512) + ch
                    pt_full = ps.tile([128, 1024], F32, tag="sc")
                    pt = pt_full[:, 0:512]
                    for d in range(DT):
                        nc.tensor.matmul(
                            pt, wsb[:, ts(d, FPC)],
                            xts_d[d][:, ts(cg, 512)],
                            start=(d == 0), stop=(d == DT - 1))
                    # ACT copies are cheap (~450ns) and ACT idles pre-attention
                    nc.scalar.copy(dst[:, ts(cg, 512)], pt)

        def v_proj(b):
            # V natural for batch b, both heads: tiles [128 tokens, 128 feats]
            for tt in range(S // 128):
                tg = b * (S // 128) + tt
                pvt_full = ps.tile([128, 1024], F32, tag="sc")
                pvt = pvt_full[:, 0:FPC]
                for d in range(DT):
                    nc.tensor.matmul(
                        pvt, xts_d[d][:, ts(tg, 128)],
                        wvs[:, ts(d, FPC)],
                        start=(d == 0), stop=(d == DT - 1))
                # one copy drops both heads into their [V|1] slots
                dst = vns_r[:, 2 * b: 2 * b + 2, tt, 0:64]
                src = pvt.rearrange("p (h c) -> p h c", c=DH)
                nc.vector.tensor_copy(dst, src)

        # deferred (software-pipelined) softmax normalization state
        pending = []

        def emit_normalize():
            if not pending:
                return
            cpt, b, j = pending.pop()
            # reciprocal of the denominator row: a [1, N] DVE reciprocal runs
            # on ONE lane at ~8 cyc/elem (7.9 us measured).  Spread the row
            # over 32 lanes with the DVE 32x32 block transpose, recip the
            # valid column of each block (~0.3 us), transpose back.
            dn32 = sm.tile([32, 2 * QT], F32, tag="dn32")
            nc.vector.tensor_copy(dn32[0:1, :], cpt[64:65, :])
            dnT = sm.tile([32, 2 * QT], F32, tag="dnT")
            nc.vector.transpose(dnT[:], dn32[:])
            rcT = sm.tile([32, 2 * QT], F16, tag="rcT")
            with nc.allow_low_precision(reason="softmax recip fp16"):
                nc.vector.reciprocal(
                    rcT[:].rearrange("p (blk c) -> p blk c", c=32)[:, :, 0:1],
                    dnT[:].rearrange("p (blk c) -> p blk c", c=32)[:, :, 0:1])
            rc32 = sm.tile([32, 2 * QT], F16, tag="rc32")
            nc.vector.transpose(rc32[:], rcT[:])
            rc = rc32[0:1, :]
            # rank-1 fp16 broadcasts of 1/denom into rows 64..127, per bank
            for hl in range(2):
                nc.tensor.matmul(cpt[64:128, ts(hl, QT)], ones64[:],
                                 rc[:, ts(hl, QT)], start=True, stop=True)
            bcs = sm.tile([64, 2 * QT], F32, tag="bcs")
            nc.vector.tensor_copy(bcs[:], cpt[64:128, :])
            ctt = sm.tile([64, 2 * QT], BF, tag="ctt")
            nc.vector.tensor_mul(ctt[:], cpt[0:64, :], bcs[:])
            for hl in range(2):
                nc.sync.dma_start(
                    a2a_in[b * NQ + j, ts(hl, DH), :], ctt[:, ts(hl, QT)])

        def attention_pair(b):
            u0, u1 = 2 * b, 2 * b + 1
            qoff = b * S
            EXPS = float(1.0 / np.sqrt(DH))

            def score_mm(dst, hl, t, j):
                fr = hl * DH
                nc.tensor.matmul(
                    dst,
                    kts[fr:fr + DH, qoff + t * 128: qoff + t * 128 + 128],
                    qts[fr:fr + DH, qoff + j * QT: qoff + (j + 1) * QT],
                    start=True, stop=True)

            for j in range(NQ):
                nkt = (j + 1) * (QT // 128)
                # work units: the 4 diagonal k-tiles individually (pair of
                # heads in one [h0|h1] slot; masked), then the mask-free
                # k-tiles in runs of 2 per head (slot A = h0, slot B = h1).
                # Score matmuls are emitted in h0/h1 bursts of one 64x128
                # tiling mode so row-tiled pairs (T0/T8) execute concurrently.
                units = [(t,) for t in range(4 * j, 4 * j + 4)]
                rest = list(range(0, 4 * j))
                while rest:
                    units.append(tuple(rest[:2]))
                    rest = rest[2:]
                cpt = pc.tile([128, 2 * QT], F32, tag="cpt")
                n_av = 0
                for ui, unit in enumerate(units):
                    diag = len(unit) == 1 and unit[0] >= 4 * j
                    if ui == 1:
                        # previous (b,j) normalization; its PE broadcasts
                        # land behind this j's early score matmuls
                        emit_normalize()
                    if diag:
                        t = unit[0]
                        o = t - 4 * j
                        sct = ps.tile([128, 2 * QT], F32, tag="sc")
                        score_mm(sct[:, 0:QT], 0, t, j)
                        score_mm(sct[:, QT:2 * QT], 1, t, j)
                        ext = work.tile([128, 2 * QT], BF, tag="exp")
                        ext_h = ext[:].rearrange("p (h q) -> p h q", h=2)
                        sct_h = sct[:].rearrange("p (h q) -> p h q", h=2)
                        if o == 0:
                            nc.scalar.activation(
                                ext[:], sct[:],
                                mybir.ActivationFunctionType.Exp, scale=EXPS)
                        else:
                            nc.gpsimd.memset(ext_h[:, :, 0:o * 128], 0.0)
                            nc.scalar.activation(
                                ext_h[:, :, o * 128:QT],
                                sct_h[:, :, o * 128:QT],
                                mybir.ActivationFunctionType.Exp, scale=EXPS)
                        dg = ext_h[:, :, o * 128:(o + 1) * 128]
                        nc.vector.tensor_mul(
                            dg, dg, mks[:].rearrange("p (h c) -> p h c", h=2))
                        for hl, u in ((0, u0), (1, u1)):
                            nc.tensor.matmul(
                                cpt[0:65, ts(hl, QT)], vn_ap(u, t),
                                ext[:, ts(hl, QT)],
                                start=(n_av == 0), stop=(n_av == nkt - 1))
                        n_av += 1
                    else:
                        n = len(unit)
                        sctA = ps.tile([128, 2 * QT], F32, tag="sc")
                        sctB = ps.tile([128, 2 * QT], F32, tag="sc")
                        for i, t in enumerate(unit):
                            score_mm(sctA[:, ts(i, QT)], 0, t, j)
                            score_mm(sctB[:, ts(i, QT)], 1, t, j)
                        extA = work.tile([128, 2 * QT], BF, tag="exp")
                        extB = work.tile([128, 2 * QT], BF, tag="exp")
                        nc.scalar.activation(
                            extA[:, 0:n * QT], sctA[:, 0:n * QT],
                            mybir.ActivationFunctionType.Exp, scale=EXPS)
                        nc.scalar.activation(
                            extB[:, 0:n * QT], sctB[:, 0:n * QT],
                            mybir.ActivationFunctionType.Exp, scale=EXPS)
                        for i, t in enumerate(unit):
                            for hl, u, ex in ((0, u0, extA), (1, u1, extB)):
                                nc.tensor.matmul(
                                    cpt[0:65, ts(hl, QT)], vn_ap(u, t),
                                    ex[:, ts(i, QT)],
                                    start=(n_av == 0),
                                    stop=(n_av == nkt - 1))
                            n_av += 1
                pending.append((cpt, b, j))

        def qkv_all():
            qkt_proj(0)
            v_proj(0)
            qkt_proj(1)
            v_proj(1)

        def attn_all():
            attention_pair(0)
            attention_pair(1)

        amp_qkv = "qkv" in phases
        amp_attn = "attn" in phases
        if not amp_qkv:
            qkv_all()
        if not amp_attn:
            attn_all()
            emit_normalize()
        for _rep in range(amp):
            if amp_qkv and amp_attn:
                # projections fully up-front: they borrow the attention's
                # sc PSUM slots, so they must not overlap the attention era
                qkt_proj(0)
                v_proj(0)
                qkt_proj(1)
                v_proj(1)
                attention_pair(0)
                attention_pair(1)
                emit_normalize()
            elif amp_qkv:
                qkv_all()
            elif amp_attn:
                attn_all()
                emit_normalize()

        # ---- AllToAll: head-sharded ctx -> token-sharded ctx
        if collective:
            nc.gpsimd.collective_compute(
                "AllToAll", mybir.AluOpType.bypass,
                replica_groups=[list(range(NCORES))],
                ins=[a2a_in.opt()], outs=[a2a_out.opt()])
        else:
            for g in range(NCORES):
                nc.sync.dma_start(a2a_out[g], a2a_in[g])

        for _rep in range(amp if "proj" in phases else 1):
            # ---- output projection for this core's token chunk
            for jb in range(NCORES):
                nc.sync.dma_start(a2s[:, ts(jb, CHUNK)], a2a_out[jb])
            for f in range(DT):
                pot_full = ps.tile([128, 1024], F32, tag="sc")
                pot = pot_full[:, 0:512]
                for jb in range(NCORES):
                    nc.tensor.matmul(
                        pot[:], wos[:, jb * D + f * 128: jb * D + (f + 1) * 128],
                        a2s[:, ts(jb, CHUNK)],
                        start=(jb == 0), stop=(jb == NCORES - 1))
                ot = sm.tile([128, CHUNK], F32, tag="ot")
                nc.vector.tensor_scalar_add(ot[:], pot[:], bos[:, f:f + 1])
                nc.sync.dma_start(out[ts(f, 128), :], ot[:])

    if compile:
        nc.compile()
    _BUILD_CACHE[key] = nc
    return nc


def _make_inputs(X, W_q, W_k, W_v, W_o, b_o):
    Xf = np.asarray(X, np.float32).reshape(T, D)
    xt = np.ascontiguousarray(Xf.T).astype(BF16)
    wo = np.ascontiguousarray(np.asarray(W_o, np.float32)).astype(BF16)
    bo = np.asarray(b_o, np.float32).reshape(D, 1)
    kk = np.arange(128)[:, None]
    qq = np.arange(128)[None, :]
    # [tri | tri]: lower-triangular keep-mask for the 128x128 diagonal
    # sub-block, doubled so one strided multiply covers both heads
    msk = np.tile((qq >= kk), (1, 2)).astype(BF16)
    in_maps = []
    for c in range(NCORES):
        sl = slice(c * FPC, (c + 1) * FPC)
        in_maps.append({
            "xt": xt,
            "wq": np.ascontiguousarray(np.asarray(W_q, np.float32)[:, sl]).astype(BF16),
            "wk": np.ascontiguousarray(np.asarray(W_k, np.float32)[:, sl]).astype(BF16),
            "wv": np.ascontiguousarray(np.asarray(W_v, np.float32)[:, sl]).astype(BF16),
            "wo": wo,
            "bo": bo,
            "msk": msk,
        })
    return in_maps


def kernel(X, W_q, W_k, W_v, W_o, b_o):
    nc = _build()
    in_maps = _make_inputs(X, W_q, W_k, W_v, W_o, b_o)
    res = run_bass_kernel_spmd(nc, in_maps, list(range(NCORES)))
    out_t = np.concatenate([res.results[c]["out"] for c in range(NCORES)],
                           axis=1)  # [D, T]
    return np.ascontiguousarray(out_t.T).reshape(B, S, D).astype(np.float32)


# revision 26
# speedup vs baseline: 9.1057x; 1.0641x over previous
"""Multi-head attention (B=2, S=2048, D=1024, H=16) on 8 Trainium2 cores.

Sharding: head-group parallel (2 heads per core) for QKV projections +
attention; an 8-rank AllToAll redistributes context from head-sharding to
token-sharding; each core then runs the output projection (full W_o) for
its 512-token chunk (global chunk index == core index).

The core's two heads run through attention as a PAIR: score matmuls
contract over DH=64, so h0 (SBUF partitions 0-63) and h1 (64-127) derive
tile_position (0,0)/(64,0) and can execute concurrently in the PE array
(row tiling) when issued back-to-back in one 64x128 tiling mode.
Projections run before/after attention and borrow the attention PSUM
slots: sc 2x[128,1024] + cpt 2x[128,1024] = all 8 banks.

Softmax: no max-subtraction (scores bounded); denominators ride as an
appended ones-column in the AV stationary operand ([V|1], M=65); the
[1,1024] denominator row is reciprocal'd by spreading it over 32 lanes
with the DVE 32x32 block transpose (a single-lane DVE reciprocal measures
~8 cyc/elem), then broadcast back over 64 partitions via rank-1 fp16
matmuls.  Causal masking: fully-masked 128-column blocks of diagonal
score tiles are skipped by exp and zero-filled by GPSIMD memsets; only
the triangular 128x128 sub-block pays a (strided, both-heads) mask
multiply on DVE.

Measured on hardware (NTFF profile, core 0): 291 us vs 366 us for the
previous baseline.
"""

import os
import sys
from contextlib import ExitStack

for _p in ("/opt/trn_rl_repo",):
    if os.path.isdir(_p) and _p not in sys.path:
        sys.path.insert(0, _p)

import numpy as np
import ml_dtypes

import concourse.bass as bass
import concourse.tile as tile
from concourse import bacc, mybir
from concourse.bass import ts
from concourse.bass_utils import run_bass_kernel_spmd

BF16 = ml_dtypes.bfloat16
BF = mybir.dt.bfloat16
F16 = mybir.dt.float16
F8 = mybir.dt.float8e4
F32 = mybir.dt.float32

B, S, D, H, DH = 2, 2048, 1024, 16, 64
NCORES = 8
T = B * S              # 4096 flattened tokens
FPC = D // NCORES      # 128 features per core (2 heads)
CHUNK = T // NCORES    # 512 tokens per core in the output phase
DT = D // 128          # 8 contraction tiles over D
QT = 512               # attention q-tile
NQ = S // QT           # 4 q-tiles per (batch, head)
NKT = S // 128         # 16 k-tiles per (batch, head)
UNITS = B * (FPC // DH)  # 4 attention units per core: (batch, local head)

_BUILD_CACHE = {}


def _build(amp=1, collective=True, num_devices=NCORES, compile=True,
           phases="qkv,attn,proj"):
    key = (amp, collective, num_devices, compile, phases)
    if key in _BUILD_CACHE:
        return _BUILD_CACHE[key]
    nc = bacc.Bacc("TRN2", target_bir_lowering=False, debug=False,
                   num_devices=num_devices)
    xt = nc.dram_tensor("xt", [D, T], BF, kind="ExternalInput").ap()
    wq = nc.dram_tensor("wq", [D, FPC], BF, kind="ExternalInput").ap()
    wk = nc.dram_tensor("wk", [D, FPC], BF, kind="ExternalInput").ap()
    wv = nc.dram_tensor("wv", [D, FPC], BF, kind="ExternalInput").ap()
    wo = nc.dram_tensor("wo", [D, D], BF, kind="ExternalInput").ap()
    bo = nc.dram_tensor("bo", [D, 1], F32, kind="ExternalInput").ap()
    msk = nc.dram_tensor("msk", [128, 256], BF, kind="ExternalInput").ap()
    out = nc.dram_tensor("out", [D, CHUNK], BF, kind="ExternalOutput").ap()

    with tile.TileContext(nc) as tc, ExitStack() as ctx:
        pers = ctx.enter_context(tc.tile_pool(name="pers", bufs=1))
        # PSUM (8 banks): sc 2x[128,1024] + cpt 2x[128,1024]; projections
        # run before/after attention and borrow the sc slots
        ps = ctx.enter_context(tc.tile_pool(name="ps", bufs=2, space="PSUM"))
        pc = ctx.enter_context(tc.tile_pool(name="pc", bufs=2, space="PSUM"))
        work = ctx.enter_context(tc.tile_pool(name="work", bufs=4))
        sm = ctx.enter_context(tc.tile_pool(name="sm", bufs=2))
        dram = ctx.enter_context(tc.tile_pool(name="dram", bufs=1, space="DRAM"))

        # ---- persistent SBUF tensors
        xts_d = []
        for d in range(DT):
            xt_t = pers.tile([128, T], BF, tag=f"xts{d}")
            xts_d.append(xt_t)
        qts = pers.tile([128, T], BF, tag="qts")
        kts = pers.tile([128, T], BF, tag="kts")
        # Vn per (unit, k-tile): [128 tokens, 65] = [V | ones]
        vns = pers.tile([128, UNITS * NKT * 65], BF, tag="vns")
        wqs = pers.tile([128, DT * FPC], BF, tag="wqs")
        wks = pers.tile([128, DT * FPC], BF, tag="wks")
        wvs = pers.tile([128, DT * FPC], BF, tag="wvs")
        wos = pers.tile([128, DT * D], BF, tag="wos")
        mks = pers.tile([128, 256], BF, tag="mks")
        bos = pers.tile([128, DT], F32, tag="bos")
        ones64 = pers.tile([1, 64], F16, tag="ones64")
        a2s = pers.tile([128, NCORES * CHUNK], BF, tag="a2s")

        def vn_ap(u, t):
            o = (u * NKT + t) * 65
            return vns[:, o:o + 65]

        # V-destination AP covering both heads of batch b, k-tile t:
        # [128, 2 heads (stride NKT*65), 64 cols]
        vns_r = vns[:].rearrange("p (u t c) -> p u t c", t=NKT, c=65)

        # ---- load weights / constants (small tensors first: queue heads)
        for wsb, wdr in ((wqs, wq), (wks, wk), (wvs, wv)):
            nc.sync.dma_start(
                wsb[:].rearrange("p (d f) -> p d f", d=DT),
                wdr.rearrange("(d p) f -> p d f", p=128))
        nc.sync.dma_start(mks[:], msk[:])
        nc.sync.dma_start(
            bos[:].rearrange("p (f o) -> p f o", o=1),
            bo.rearrange("(f p) o -> p f o", p=128))
        # X^T: 8 sub-DMAs per d-tile to spread over the DGE queues.
        # Sub-chunk-major order: the first projection chunk needs sub-chunk 0
        # of ALL d-tiles, so issuing c8-outer unblocks it ~8x earlier.
        for c8 in range(8):
            for d in range(DT):
                nc.sync.dma_start(xts_d[d][:, ts(c8, T // 8)],
                                  xt[ts(d, 128), ts(c8, T // 8)])
        for jb in range(DT):
            nc.sync.dma_start(wos[:, ts(jb, D)], wo[ts(jb, 128), :])
        nc.vector.memset(ones64[:], 1.0)
        nc.vector.memset(
            vns[:].rearrange("p (n c) -> p n c", c=65)[:, :, 64:65], 1.0)

        a2a_in = dram.tile([NCORES, FPC, CHUNK], BF, tag="a2a_in")
        a2a_out = dram.tile([NCORES, FPC, CHUNK], BF, tag="a2a_out")

        def qkt_proj(b):
            # Q^T, K^T for batch b: [128 feats, tokens b*S .. b*S+S]
            for wsb, dst in ((wqs, qts), (wks, kts)):
                for ch in range(S // 512):
                    cg = b * (S // 512) + ch
                    pt_full = ps.tile([128, 1024], F32, tag="sc")
                    pt = pt_full[:, 0:512]
                    for d in range(DT):
                        nc.tensor.matmul(
                            pt, wsb[:, ts(d, FPC)],
                            xts_d[d][:, ts(cg, 512)],
                            start=(d == 0), stop=(d == DT - 1))
                    # ACT copies are cheap (~450ns) and ACT idles pre-attention
                    nc.scalar.copy(dst[:, ts(cg, 512)], pt)

        def v_proj(b):
            # V natural for batch b, both heads: tiles [128 tokens, 128 feats]
            for tt in range(S // 128):
                tg = b * (S // 128) + tt
                pvt_full = ps.tile([128, 1024], F32, tag="sc")
                pvt = pvt_full[:, 0:FPC]
                for d in range(DT):
                    nc.tensor.matmul(
                        pvt, xts_d[d][:, ts(tg, 128)],
                        wvs[:, ts(d, FPC)],
                        start=(d == 0), stop=(d == DT - 1))
                # one copy drops both heads into their [V|1] slots
                dst = vns_r[:, 2 * b: 2 * b + 2, tt, 0:64]
                src = pvt.rearrange("p (h c) -> p h c", c=DH)
                nc.vector.tensor_copy(dst, src)

        # deferred (software-pipelined) softmax normalization state
        pending = []

        def emit_normalize():
            if not pending:
                return
            cpt, b, j = pending.pop()
            # reciprocal of the denominator row: a [1, N] DVE reciprocal runs
            # on ONE lane at ~8 cyc/elem (7.9 us measured).  Spread the row
            # over 32 lanes with the DVE 32x32 block transpose, recip the
            # valid column of each block (~0.3 us), transpose back.
            dn32 = sm.tile([32, 2 * QT], F32, tag="dn32")
            nc.vector.tensor_copy(dn32[0:1, :], cpt[64:65, :])
            dnT = sm.tile([32, 2 * QT], F32, tag="dnT")
            nc.vector.transpose(dnT[:], dn32[:])
            rcT = sm.tile([32, 2 * QT], F16, tag="rcT")
            with nc.allow_low_precision(reason="softmax recip fp16"):
                nc.vector.reciprocal(
                    rcT[:].rearrange("p (blk c) -> p blk c", c=32)[:, :, 0:1],
                    dnT[:].rearrange("p (blk c) -> p blk c", c=32)[:, :, 0:1])
            rc32 = sm.tile([32, 2 * QT], F16, tag="rc32")
            nc.vector.transpose(rc32[:], rcT[:])
            rc = rc32[0:1, :]
            # rank-1 fp16 broadcasts of 1/denom into rows 64..127, per bank
            for hl in range(2):
                nc.tensor.matmul(cpt[64:128, ts(hl, QT)], ones64[:],
                                 rc[:, ts(hl, QT)], start=True, stop=True)
            bcs = sm.tile([64, 2 * QT], F32, tag="bcs")
            nc.vector.tensor_copy(bcs[:], cpt[64:128, :])
            ctt = sm.tile([64, 2 * QT], BF, tag="ctt")
            nc.vector.tensor_mul(ctt[:], cpt[0:64, :], bcs[:])
            for hl in range(2):
                nc.sync.dma_start(
                    a2a_in[b * NQ + j, ts(hl, DH), :], ctt[:, ts(hl, QT)])

        def attention_pair(b):
            u0, u1 = 2 * b, 2 * b + 1
            qoff = b * S
            EXPS = float(1.0 / np.sqrt(DH))

            def score_mm(dst, hl, t, j):
                fr = hl * DH
                nc.tensor.matmul(
                    dst,
                    kts[fr:fr + DH, qoff + t * 128: qoff + t * 128 + 128],
                    qts[fr:fr + DH, qoff + j * QT: qoff + (j + 1) * QT],
                    start=True, stop=True)

            for j in range(NQ):
                nkt = (j + 1) * (QT // 128)
                # work units: the 4 diagonal k-tiles individually (pair of
                # heads in one [h0|h1] slot; masked), then the mask-free
                # k-tiles in runs of 2 per head (slot A = h0, slot B = h1).
                # Score matmuls are emitted in h0/h1 bursts of one 64x128
                # tiling mode so row-tiled pairs (T0/T8) execute concurrently.
                units = [(t,) for t in range(4 * j, 4 * j + 4)]
                rest = list(range(0, 4 * j))
                while rest:
                    units.append(tuple(rest[:2]))
                    rest = rest[2:]
                cpt = pc.tile([128, 2 * QT], F32, tag="cpt")
                n_av = 0
                # AV matmuls are emitted one unit LATE (after the next
                # unit's score burst) so the scheduler sees longer unbroken
                # 64x128-mode score stretches -> more T0/T8 pair packing.
                deferred_av = []
                for ui, unit in enumerate(units):
                    diag = len(unit) == 1 and unit[0] >= 4 * j
                    if diag:
                        t = unit[0]
                        o = t - 4 * j
                        sct = ps.tile([128, 2 * QT], F32, tag="sc")
                        score_mm(sct[:, 0:QT], 0, t, j)
                        score_mm(sct[:, QT:2 * QT], 1, t, j)
                    else:
                        n = len(unit)
                        sctA = ps.tile([128, 2 * QT], F32, tag="sc")
                        sctB = ps.tile([128, 2 * QT], F32, tag="sc")
                        for i, t in enumerate(unit):
                            score_mm(sctA[:, ts(i, QT)], 0, t, j)
                            score_mm(sctB[:, ts(i, QT)], 1, t, j)
                    if ui == 1:
                        # previous (b,j) normalization; its PE broadcasts
                        # land behind this j's early score matmuls
                        emit_normalize()
                    for av in deferred_av:
                        av()
                    deferred_av = []
                    if diag:
                        ext = work.tile([128, 2 * QT], BF, tag="exp")
                        ext_h = ext[:].rearrange("p (h q) -> p h q", h=2)
                        sct_h = sct[:].rearrange("p (h q) -> p h q", h=2)
                        if o == 0:
                            nc.scalar.activation(
                                ext[:], sct[:],
                                mybir.ActivationFunctionType.Exp, scale=EXPS)
                        else:
                            nc.gpsimd.memset(ext_h[:, :, 0:o * 128], 0.0)
                            nc.scalar.activation(
                                ext_h[:, :, o * 128:QT],
                                sct_h[:, :, o * 128:QT],
                                mybir.ActivationFunctionType.Exp, scale=EXPS)
                        dg = ext_h[:, :, o * 128:(o + 1) * 128]
                        nc.vector.tensor_mul(
                            dg, dg, mks[:].rearrange("p (h c) -> p h c", h=2))

                        def av_diag(t=t, ext=ext, na=n_av):
                            for hl, u in ((0, u0), (1, u1)):
                                nc.tensor.matmul(
                                    cpt[0:65, ts(hl, QT)], vn_ap(u, t),
                                    ext[:, ts(hl, QT)],
                                    start=(na == 0), stop=(na == nkt - 1))
                        deferred_av.append(av_diag)
                        n_av += 1
                    else:
                        extA = work.tile([128, 2 * QT], BF, tag="exp")
                        extB = work.tile([128, 2 * QT], BF, tag="exp")
                        nc.scalar.activation(
                            extA[:, 0:n * QT], sctA[:, 0:n * QT],
                            mybir.ActivationFunctionType.Exp, scale=EXPS)
                        nc.scalar.activation(
                            extB[:, 0:n * QT], sctB[:, 0:n * QT],
                            mybir.ActivationFunctionType.Exp, scale=EXPS)

                        def av_pair(unit=unit, extA=extA, extB=extB,
                                    na=n_av):
                            k = na
                            for i, t in enumerate(unit):
                                for hl, u, ex in ((0, u0, extA),
                                                  (1, u1, extB)):
                                    nc.tensor.matmul(
                                        cpt[0:65, ts(hl, QT)], vn_ap(u, t),
                                        ex[:, ts(i, QT)],
                                        start=(k == 0),
                                        stop=(k == nkt - 1))
                                k += 1
                        deferred_av.append(av_pair)
                        n_av += len(unit)
                for av in deferred_av:
                    av()
                pending.append((cpt, b, j))

        def qkv_all():
            qkt_proj(0)
            v_proj(0)
            qkt_proj(1)
            v_proj(1)

        def attn_all():
            attention_pair(0)
            attention_pair(1)

        amp_qkv = "qkv" in phases
        amp_attn = "attn" in phases
        if not amp_qkv:
            qkv_all()
        if not amp_attn:
            attn_all()
            emit_normalize()
        for _rep in range(amp):
            if amp_qkv and amp_attn:
                # projections fully up-front: they borrow the attention's
                # sc PSUM slots, so they must not overlap the attention era
                qkt_proj(0)
                v_proj(0)
                qkt_proj(1)
                v_proj(1)
                attention_pair(0)
                attention_pair(1)
                emit_normalize()
            elif amp_qkv:
                qkv_all()
            elif amp_attn:
                attn_all()
                emit_normalize()

        # ---- AllToAll: head-sharded ctx -> token-sharded ctx
        if collective:
            nc.gpsimd.collective_compute(
                "AllToAll", mybir.AluOpType.bypass,
                replica_groups=[list(range(NCORES))],
                ins=[a2a_in.opt()], outs=[a2a_out.opt()])
        else:
            for g in range(NCORES):
                nc.sync.dma_start(a2a_out[g], a2a_in[g])

        for _rep in range(amp if "proj" in phases else 1):
            # ---- output projection for this core's token chunk
            for jb in range(NCORES):
                nc.sync.dma_start(a2s[:, ts(jb, CHUNK)], a2a_out[jb])
            for f in range(DT):
                pot_full = ps.tile([128, 1024], F32, tag="sc")
                pot = pot_full[:, 0:512]
                for jb in range(NCORES):
                    nc.tensor.matmul(
                        pot[:], wos[:, jb * D + f * 128: jb * D + (f + 1) * 128],
                        a2s[:, ts(jb, CHUNK)],
                        start=(jb == 0), stop=(jb == NCORES - 1))
                ot = sm.tile([128, CHUNK], BF, tag="ot")
                nc.vector.tensor_scalar_add(ot[:], pot[:], bos[:, f:f + 1])
                nc.sync.dma_start(out[ts(f, 128), :], ot[:])

    if compile:
        nc.compile()
    _BUILD_CACHE[key] = nc
    return nc


def _make_inputs(X, W_q, W_k, W_v, W_o, b_o):
    Xf = np.asarray(X, np.float32).reshape(T, D)
    xt = np.ascontiguousarray(Xf.T).astype(BF16)
    wo = np.ascontiguousarray(np.asarray(W_o, np.float32)).astype(BF16)
    bo = np.asarray(b_o, np.float32).reshape(D, 1)
    kk = np.arange(128)[:, None]
    qq = np.arange(128)[None, :]
    # [tri | tri]: lower-triangular keep-mask for the 128x128 diagonal
    # sub-block, doubled so one strided multiply covers both heads
    msk = np.tile((qq >= kk), (1, 2)).astype(BF16)
    in_maps = []
    for c in range(NCORES):
        sl = slice(c * FPC, (c + 1) * FPC)
        in_maps.append({
            "xt": xt,
            "wq": np.ascontiguousarray(np.asarray(W_q, np.float32)[:, sl]).astype(BF16),
            "wk": np.ascontiguousarray(np.asarray(W_k, np.float32)[:, sl]).astype(BF16),
            "wv": np.ascontiguousarray(np.asarray(W_v, np.float32)[:, sl]).astype(BF16),
            "wo": wo,
            "bo": bo,
            "msk": msk,
        })
    return in_maps


def kernel(X, W_q, W_k, W_v, W_o, b_o):
    nc = _build()
    in_maps = _make_inputs(X, W_q, W_k, W_v, W_o, b_o)
    res = run_bass_kernel_spmd(nc, in_maps, list(range(NCORES)))
    out_t = np.concatenate([res.results[c]["out"] for c in range(NCORES)],
                           axis=1)  # [D, T]
    return np.ascontiguousarray(out_t.T).reshape(B, S, D).astype(np.float32)
